# revision 6
# baseline (speedup 1.0000x reference)
"""VQ-codebook 3x3 conv (nn_CConv) on 8 Trainium2 NeuronCores.

Sharding: data-parallel over the batch (16 images -> 2 per core); the small
codebook-derived weights / scales / bias are replicated to every core.
Host-side work is layout only: batch split, reshape/transpose of the index
and scale matrices, and the codebook row gather (pure indexing, no
arithmetic).

Per-core device program (one NEFF, SPMD over 8 cores):
  - weight build (on device): fp16 round-trip of scales (dequant emulation),
    multiply by cut, broadcast-multiply onto the gathered codebook rows;
    weights stored k-major so each of the 9 taps is a contiguous
    [128(in), 128(out)] fp16 stationary block.
  - conv: each image is zero-padded to rows of width 114 in SBUF (borders
    zeroed on-chip); the 3x3 conv is 9 accumulating PE matmuls over shifted
    views of the flattened padded image, fp16 in / fp32 PSUM accumulate.
    Outputs are computed in "padded q space" (q = h*114 + w); junk columns
    w in {112,113} are computed but dropped by the strided output DMA.
  - images are processed in row-slabs (8/20/28 output rows; tiny slabs at the
    kernel's two ends shorten the serial prologue and the final-DMA tail);
    input loads (sync HWDGE queue) are double-buffered and cast f32->f16 by
    the scalar engine; PSUM is evacuated with a fused per-partition bias add
    on the vector engine; output DMAs ride the scalar HWDGE queue so they
    never head-of-line-block input loads.
  - 14 dummy warm-up matmuls run during the prologue so the PE HAM clock
    gate reaches 2.4 GHz before the real matmuls start.
"""
import sys
import types
from contextlib import ExitStack

import numpy as np

import concourse.tile as tile
from concourse import bacc, mybir


def _ensure_axon_hooks_module():
    """This image's antenv package lacks axon_hooks; bass_utils imports it
    when tracing is requested (e.g. BASS_TRACE=1). Provide a no-op shim."""
    try:
        import antenv

        if "antenv.axon_hooks" not in sys.modules and not hasattr(
            antenv, "axon_hooks"
        ):
            mod = types.ModuleType("antenv.axon_hooks")
            holder = [None]
            mod.set_axon_ntff_profile_hook = lambda h: holder.__setitem__(0, h)
            mod.get_axon_ntff_profile_hook = lambda: holder[0]
            antenv.axon_hooks = mod
            sys.modules["antenv.axon_hooks"] = mod
    except Exception:
        pass


_ensure_axon_hooks_module()

from concourse import bass_utils  # noqa: E402

P = 128
H = W = 112
WP = 114
IMGS = 2
N_CORES = 8

f32 = mybir.dt.float32
f16 = mybir.dt.float16

SLAB_PLAN = {0: [8, 20, 28, 28, 28], 1: [28, 28, 28, 24, 4]}
MAX_SO = 28
WARM_MMS = 7

_CACHE = {}


def _slab_tiles(slab_out):
    n_pos = slab_out * WP - 2
    full, r = divmod(n_pos, 512)
    tiles = [512] * full
    if r:
        if r < 256 and full:
            tiles = [512] * (full - 1) + [(512 + r) // 2, (512 + r) - (512 + r) // 2]
        else:
            tiles.append(r)
    assert sum(tiles) == n_pos
    return tiles


def _build():
    nc = bacc.Bacc("TRN2", target_bir_lowering=False, debug=False)

    x_t = nc.dram_tensor("x", [IMGS, P, H, W], f32, kind="ExternalInput")
    scalesT_t = nc.dram_tensor("scalesT", [P, P], f32, kind="ExternalInput")
    cutT_t = nc.dram_tensor("cutT", [P, P], f32, kind="ExternalInput")
    bias_t = nc.dram_tensor("bias", [P, 1], f32, kind="ExternalInput")
    wrawT_t = nc.dram_tensor("wrawT", [P, P * 9], f32, kind="ExternalInput")
    # fp16 output: halves the store traffic; host widens to f32 (lossless)
    out_t = nc.dram_tensor("out", [IMGS, P, H, W], f16, kind="ExternalOutput")

    with tile.TileContext(nc) as tc, ExitStack() as ctx:
        wb = ctx.enter_context(tc.tile_pool(name="wb", bufs=1))
        xp = ctx.enter_context(tc.tile_pool(name="xp", bufs=4))
        op = ctx.enter_context(tc.tile_pool(name="op", bufs=4))
        ps = ctx.enter_context(tc.tile_pool(name="ps", bufs=6, space="PSUM"))
        xs = ctx.enter_context(tc.tile_pool(name="xs", bufs=4))

        # Small weight-side DMAs head the sync queue: the weight build is the
        # critical path to the first real matmul. w_raw (the big one) follows
        # the tiny scales/cut/bias transfers; the slab-0 input load rides the
        # otherwise-idle scalar HWDGE queue in parallel.
        sc_in = wb.tile([P, P], f32, tag="sc_in")
        nc.sync.dma_start(sc_in[:], scalesT_t.ap())
        cut_s = wb.tile([P, P], f32, tag="cut")
        nc.sync.dma_start(cut_s[:], cutT_t.ap())
        bias_s = wb.tile([P, 1], f32, tag="bias")
        nc.sync.dma_start(bias_s[:], bias_t.ap())
        w_raw = wb.tile([P, P * 9], f32, tag="w_raw")
        nc.sync.dma_start(w_raw[:], wrawT_t.ap())

        so0 = SLAB_PLAN[0][0]
        nrows0 = min(H, so0 + 1)
        pre_stage = xs.tile([P, (MAX_SO + 2) * W], f32, tag="xstage")
        nc.scalar.dma_start(pre_stage[:, :nrows0 * W], x_t.ap()[0, :, 0:nrows0, :])

        # PE warmup: HAM un-throttles to 2.4 GHz during the prologue. Sized so
        # the last warmup ends right as the weights/slab-0 become ready — any
        # PE idle gap makes HAM re-throttle to half rate for a ~3.4us window.
        wrm = wb.tile([P, 512], f16, tag="warm")
        nc.vector.memset(wrm[:], 0.0)
        pw = ps.tile([P, 512], f32, tag="pst")
        for _ in range(WARM_MMS):
            nc.tensor.matmul(pw[:], wrm[:, :P], wrm[:], start=True, stop=True)

        # ---- weight build ----
        sc16 = wb.tile([P, P], f16, tag="sc16")
        nc.vector.tensor_copy(sc16[:], sc_in[:])
        sc = wb.tile([P, P], f32, tag="sc")
        nc.vector.tensor_copy(sc[:], sc16[:])
        scc = wb.tile([P, P], f32, tag="scc")
        nc.vector.tensor_tensor(
            out=scc[:], in0=sc[:], in1=cut_s[:], op=mybir.AluOpType.mult
        )

        # w_mm[i, k, o] = w_raw[i, o, k] * scc[i, o]
        w_mm = wb.tile([P, 9 * P], f16, tag="w_mm")
        w_raw3 = w_raw[:].rearrange("p (o k) -> p k o", k=9)
        scc3 = scc[:].rearrange("p (one o) -> p one o", one=1).to_broadcast(
            [P, 9, P]
        )
        w_mm3 = w_mm[:].rearrange("p (k o) -> p k o", o=P)
        nc.vector.tensor_tensor(
            out=w_mm3, in0=w_raw3, in1=scc3, op=mybir.AluOpType.mult
        )
        w_k_view = w_mm[:].rearrange("p (k o) -> p k o", o=P)

        # ---- conv slabs ----
        max_xpad_len = (MAX_SO + 2) * WP
        max_oslab_len = MAX_SO * WP
        max_stage = (MAX_SO + 2) * W
        for img in range(IMGS):
            h0 = 0
            for so in SLAB_PLAN[img]:
                slab_in = so + 2
                xpad_len = slab_in * WP
                xpad = xp.tile([P, max_xpad_len], f16, tag="xpad")
                xpad3 = xpad[:, :xpad_len].rearrange("p (r c) -> p r c", c=WP)
                # zero borders: cols {0,113} every row; pad row at image edge
                nc.gpsimd.memset(xpad3[:, :, 0:114:113], 0.0)
                if h0 == 0:
                    nc.gpsimd.memset(xpad[:, 0:WP], 0.0)
                elif h0 + so == H:
                    nc.gpsimd.memset(xpad[:, (slab_in - 1) * WP:xpad_len], 0.0)
                # interior rows: f32 staged load, scalar-engine cast to f16
                r_lo = max(0, h0 - 1)
                r_hi = min(H, h0 + so + 1)
                j0 = r_lo - (h0 - 1)
                nrows = r_hi - r_lo
                if img == 0 and h0 == 0:
                    stage = pre_stage
                else:
                    stage = xs.tile([P, max_stage], f32, tag="xstage")
                    nc.sync.dma_start(
                        stage[:, :nrows * W], x_t.ap()[img, :, r_lo:r_hi, :]
                    )
                nc.scalar.copy(
                    xpad3[:, j0:j0 + nrows, 1:1 + W],
                    stage[:, :nrows * W].rearrange("p (r c) -> p r c", c=W),
                )

                oslab = op.tile([P, max_oslab_len], f16, tag="oslab")
                q0 = 0
                for n in _slab_tiles(so):
                    pst = ps.tile([P, 512], f32, tag="pst")
                    for k in range(9):
                        dh, dw = divmod(k, 3)
                        off = q0 + dh * WP + dw
                        nc.tensor.matmul(
                            pst[:, :n],
                            w_k_view[:, k, :],
                            xpad[:, off:off + n],
                            start=(k == 0),
                            stop=(k == 8),
                        )
                    nc.vector.tensor_scalar_add(
                        oslab[:, q0:q0 + n], pst[:, :n], bias_s[:, 0:1]
                    )
                    q0 += n

                osrc = oslab[:, :so * WP].rearrange("p (r c) -> p r c", c=WP)[:, :, 0:W]
                nc.scalar.dma_start(out_t.ap()[img, :, h0:h0 + so, :], osrc)
                h0 += so

    nc.compile()
    return nc


def _make_in_maps(inputs):
    x = np.ascontiguousarray(np.asarray(inputs["x"], dtype=np.float32))
    cent = np.asarray(inputs["centroids"], dtype=np.float32).reshape(512, 9)
    idxT = np.asarray(inputs["idx"]).reshape(P, P).T          # [i, o]
    scalesT = np.ascontiguousarray(
        np.asarray(inputs["scales"], dtype=np.float32).reshape(P, P).T
    )
    cutT = np.ascontiguousarray(
        np.asarray(inputs["cut"], dtype=np.float32).reshape(P, P).T
    )
    bias = np.ascontiguousarray(
        np.asarray(inputs["bias"], dtype=np.float32).reshape(P, 1)
    )
    wrawT = np.ascontiguousarray(cent[idxT].reshape(P, P * 9))

    base = {"scalesT": scalesT, "cutT": cutT, "bias": bias, "wrawT": wrawT}
    maps = []
    for c in range(N_CORES):
        m = dict(base)
        m["x"] = np.ascontiguousarray(x[IMGS * c:IMGS * (c + 1)])
        maps.append(m)
    return maps


def _get_nc():
    if "nc" not in _CACHE:
        _CACHE["nc"] = _build()
    return _CACHE["nc"]


def _run(inputs, trace=False):
    nc = _get_nc()
    in_maps = _make_in_maps(inputs)
    res = bass_utils.run_bass_kernel_spmd(
        nc, in_maps, core_ids=list(range(N_CORES)), trace=trace
    )
    out = np.concatenate([res.results[c]["out"] for c in range(N_CORES)], axis=0)
    out = out.astype(np.float32)  # widen fp16 device output (lossless)
    return out, res


def kernel(**inputs) -> np.ndarray:
    out, _ = _run(inputs, trace=False)
    return out



# revision 7
# speedup vs baseline: 1.0339x; 1.0339x over previous
"""VQ-codebook 3x3 conv (nn_CConv) on 8 Trainium2 NeuronCores.

Sharding: data-parallel over the batch (16 images -> 2 per core); the small
codebook-derived weights / scales / bias are replicated to every core.
Host-side work is layout only: batch split, reshape/transpose of the index
and scale matrices, and the codebook row gather (pure indexing, no
arithmetic).

Per-core device program (one NEFF, SPMD over 8 cores):
  - weight build (on device): fp16 round-trip of scales (dequant emulation),
    multiply by cut, then 9 per-tap [128in x 128out] multiplies of the
    k-major gathered codebook rows into 9 separate fp16 stationary tiles --
    per-tap tiles let the first conv matmuls start as soon as tap 0 is
    ready instead of waiting for the whole weight tensor.
  - conv: images are zero-padded to rows of PITCH 113 in SBUF: each row is
    [pad | x0..x111], so the left-pad zero of row r+1 doubles as the
    right-pad zero of row r (one junk column per row instead of two).
    The 3x3 conv is 9 accumulating PE matmuls over shifted views of the
    flattened padded image, fp16 in / fp32 PSUM accumulate. Junk outputs at
    w=112 of each row are computed but dropped by the strided output DMA.
  - images are processed in row-slabs; image 0 uses a graduated ramp
    (8,8,12,...) so the PE can start while the input-DMA pipeline fills,
    image 1 ends with a tiny 4-row slab to shorten the final-DMA tail.
    Input loads ride the sync HWDGE queue and are cast f32->f16 by the
    scalar engine; PSUM is evacuated with a fused per-partition bias add on
    the vector engine into fp16 (host widens the output to f32, lossless);
    output DMAs ride the scalar queue except the last two slabs, which use
    the (by-then idle) sync queue to dodge head-of-line blocking.
  - warm-up matmuls run during the prologue so the HAM clock gate reaches
    2.4 GHz before the real matmuls start; any PE idle gap makes HAM
    re-throttle to half rate for a ~3.4us window, so the warmup count is
    sized to end right as slab 0 and the weights become ready.
"""
import sys
import types
from contextlib import ExitStack

import numpy as np

import concourse.tile as tile
from concourse import bacc, mybir


def _ensure_axon_hooks_module():
    """This image's antenv package lacks axon_hooks; bass_utils imports it
    when tracing is requested (e.g. BASS_TRACE=1). Provide a no-op shim."""
    try:
        import antenv

        if "antenv.axon_hooks" not in sys.modules and not hasattr(
            antenv, "axon_hooks"
        ):
            mod = types.ModuleType("antenv.axon_hooks")
            holder = [None]
            mod.set_axon_ntff_profile_hook = lambda h: holder.__setitem__(0, h)
            mod.get_axon_ntff_profile_hook = lambda: holder[0]
            antenv.axon_hooks = mod
            sys.modules["antenv.axon_hooks"] = mod
    except Exception:
        pass


_ensure_axon_hooks_module()

from concourse import bass_utils  # noqa: E402

P = 128
H = W = 112
WP = 113
IMGS = 2
N_CORES = 8

f32 = mybir.dt.float32
f16 = mybir.dt.float16

SLAB_PLAN = {0: [8, 8, 12, 16, 20, 24, 24], 1: [28, 28, 28, 24, 4]}
MAX_SO = 28
WARM_MMS = 9

_CACHE = {}


def _slab_tiles(slab_out):
    n_pos = slab_out * WP - 1
    full, r = divmod(n_pos, 512)
    tiles = [512] * full
    if r:
        if r < 256 and full:
            tiles = [512] * (full - 1) + [(512 + r) // 2, (512 + r) - (512 + r) // 2]
        else:
            tiles.append(r)
    assert sum(tiles) == n_pos
    return tiles


def _build():
    nc = bacc.Bacc("TRN2", target_bir_lowering=False, debug=False)

    x_t = nc.dram_tensor("x", [IMGS, P, H, W], f32, kind="ExternalInput")
    scalesT_t = nc.dram_tensor("scalesT", [P, P], f32, kind="ExternalInput")
    cutT_t = nc.dram_tensor("cutT", [P, P], f32, kind="ExternalInput")
    bias_t = nc.dram_tensor("bias", [P, 1], f32, kind="ExternalInput")
    # k-major: wrawT[i, k*128 + o] = centroids[idx[o, i], k]
    wrawT_t = nc.dram_tensor("wrawT", [P, P * 9], f32, kind="ExternalInput")
    # fp16 output: halves the store traffic; host widens to f32 (lossless)
    out_t = nc.dram_tensor("out", [IMGS, P, H, W], f16, kind="ExternalOutput")

    with tile.TileContext(nc) as tc, ExitStack() as ctx:
        wb = ctx.enter_context(tc.tile_pool(name="wb", bufs=1))
        xp = ctx.enter_context(tc.tile_pool(name="xp", bufs=4))
        op = ctx.enter_context(tc.tile_pool(name="op", bufs=4))
        ps = ctx.enter_context(tc.tile_pool(name="ps", bufs=6, space="PSUM"))
        xs = ctx.enter_context(tc.tile_pool(name="xs", bufs=4))

        # Sync-queue order: w_raw (critical path to tap 0) first, then the
        # tiny scales/cut/bias; slab-0 input rides the idle scalar queue.
        w_raw = wb.tile([P, P * 9], f32, tag="w_raw")
        nc.sync.dma_start(w_raw[:], wrawT_t.ap())
        sc_in = wb.tile([P, P], f32, tag="sc_in")
        nc.sync.dma_start(sc_in[:], scalesT_t.ap())
        cut_s = wb.tile([P, P], f32, tag="cut")
        nc.sync.dma_start(cut_s[:], cutT_t.ap())
        bias_s = wb.tile([P, 1], f32, tag="bias")
        nc.sync.dma_start(bias_s[:], bias_t.ap())

        # slab-0 input load, split in two so the first cast can start earlier
        so0 = SLAB_PLAN[0][0]
        nrows0 = min(H, so0 + 1)
        n0a = 6
        n0b = nrows0 - n0a
        pre_a = xs.tile([P, n0a * W], f32, tag="xstage_a")
        nc.scalar.dma_start(pre_a[:], x_t.ap()[0, :, 0:n0a, :])
        pre_b = xs.tile([P, n0b * W], f32, tag="xstage_b")
        nc.scalar.dma_start(pre_b[:], x_t.ap()[0, :, n0a:nrows0, :])

        # PE warmup (HAM clock ramp); warm tile memset on the vector engine
        wrm = wb.tile([P, 512], f16, tag="warm")
        nc.vector.memset(wrm[:], 0.0)
        pw = ps.tile([P, 512], f32, tag="pst")
        for _ in range(WARM_MMS):
            nc.tensor.matmul(pw[:], wrm[:, :P], wrm[:], start=True, stop=True)

        # ---- weight build ----
        sc16 = wb.tile([P, P], f16, tag="sc16")
        nc.vector.tensor_copy(sc16[:], sc_in[:])
        sc = wb.tile([P, P], f32, tag="sc")
        nc.vector.tensor_copy(sc[:], sc16[:])
        scc = wb.tile([P, P], f32, tag="scc")
        nc.vector.tensor_tensor(
            out=scc[:], in0=sc[:], in1=cut_s[:], op=mybir.AluOpType.mult
        )
        # per-tap stationary tiles: w_k[k][i, o] = w_raw[i, k, o] * scc[i, o]
        w_k = []
        for k in range(9):
            wk = wb.tile([P, P], f16, tag=f"w_k{k}")
            nc.vector.tensor_tensor(
                out=wk[:], in0=w_raw[:, k * P:(k + 1) * P], in1=scc[:],
                op=mybir.AluOpType.mult,
            )
            w_k.append(wk)

        # ---- conv slabs ----
        max_xpad_len = (MAX_SO + 2) * WP + 1
        max_oslab_len = MAX_SO * WP
        max_stage = (MAX_SO + 2) * W
        n_slabs_total = sum(len(v) for v in SLAB_PLAN.values())
        slab_idx = 0
        for img in range(IMGS):
            h0 = 0
            for so in SLAB_PLAN[img]:
                slab_in = so + 2
                xpad_len = slab_in * WP + 1
                xpad = xp.tile([P, max_xpad_len], f16, tag="xpad")
                xpad3 = xpad[:, :slab_in * WP].rearrange("p (r c) -> p r c", c=WP)
                # zero borders: left-pad col of every row + trailing guard
                # element (read as the right pad of the very last position)
                nc.gpsimd.memset(xpad3[:, :, 0:1], 0.0)
                nc.gpsimd.memset(xpad[:, xpad_len - 1:xpad_len], 0.0)
                if h0 == 0:
                    nc.gpsimd.memset(xpad[:, 0:WP], 0.0)
                elif h0 + so == H:
                    nc.gpsimd.memset(xpad[:, (slab_in - 1) * WP:xpad_len - 1], 0.0)
                # interior rows: f32 staged load, scalar-engine cast to f16
                r_lo = max(0, h0 - 1)
                r_hi = min(H, h0 + so + 1)
                j0 = r_lo - (h0 - 1)
                nrows = r_hi - r_lo
                if img == 0 and h0 == 0:
                    nc.scalar.copy(
                        xpad3[:, j0:j0 + n0a, 1:1 + W],
                        pre_a[:].rearrange("p (r c) -> p r c", c=W),
                    )
                    nc.scalar.copy(
                        xpad3[:, j0 + n0a:j0 + nrows, 1:1 + W],
                        pre_b[:].rearrange("p (r c) -> p r c", c=W),
                    )
                else:
                    stage = xs.tile([P, max_stage], f32, tag="xstage")
                    nc.sync.dma_start(
                        stage[:, :nrows * W], x_t.ap()[img, :, r_lo:r_hi, :]
                    )
                    nc.scalar.copy(
                        xpad3[:, j0:j0 + nrows, 1:1 + W],
                        stage[:, :nrows * W].rearrange("p (r c) -> p r c", c=W),
                    )

                oslab = op.tile([P, max_oslab_len], f16, tag="oslab")
                q0 = 0
                for n in _slab_tiles(so):
                    pst = ps.tile([P, 512], f32, tag="pst")
                    for k in range(9):
                        dh, dw = divmod(k, 3)
                        off = q0 + dh * WP + dw
                        nc.tensor.matmul(
                            pst[:, :n],
                            w_k[k][:],
                            xpad[:, off:off + n],
                            start=(k == 0),
                            stop=(k == 8),
                        )
                    nc.vector.tensor_scalar_add(
                        oslab[:, q0:q0 + n], pst[:, :n], bias_s[:, 0:1]
                    )
                    q0 += n

                osrc = oslab[:, :so * WP].rearrange("p (r c) -> p r c", c=WP)[:, :, 0:W]
                # last two slabs: output DMA on the (idle by now) sync queue
                if slab_idx >= n_slabs_total - 2:
                    nc.sync.dma_start(out_t.ap()[img, :, h0:h0 + so, :], osrc)
                else:
                    nc.scalar.dma_start(out_t.ap()[img, :, h0:h0 + so, :], osrc)
                h0 += so
                slab_idx += 1

    nc.compile()
    return nc


def _make_in_maps(inputs):
    x = np.ascontiguousarray(np.asarray(inputs["x"], dtype=np.float32))
    cent = np.asarray(inputs["centroids"], dtype=np.float32).reshape(512, 9)
    idxT = np.asarray(inputs["idx"]).reshape(P, P).T          # [i, o]
    scalesT = np.ascontiguousarray(
        np.asarray(inputs["scales"], dtype=np.float32).reshape(P, P).T
    )
    cutT = np.ascontiguousarray(
        np.asarray(inputs["cut"], dtype=np.float32).reshape(P, P).T
    )
    bias = np.ascontiguousarray(
        np.asarray(inputs["bias"], dtype=np.float32).reshape(P, 1)
    )
    # [i, o, k] -> k-major [i, k, o] (pure layout)
    wrawT = np.ascontiguousarray(
        cent[idxT].transpose(0, 2, 1).reshape(P, P * 9)
    )

    base = {"scalesT": scalesT, "cutT": cutT, "bias": bias, "wrawT": wrawT}
    maps = []
    for c in range(N_CORES):
        m = dict(base)
        m["x"] = np.ascontiguousarray(x[IMGS * c:IMGS * (c + 1)])
        maps.append(m)
    return maps


def _get_nc():
    if "nc" not in _CACHE:
        _CACHE["nc"] = _build()
    return _CACHE["nc"]


def _run(inputs, trace=False):
    nc = _get_nc()
    in_maps = _make_in_maps(inputs)
    res = bass_utils.run_bass_kernel_spmd(
        nc, in_maps, core_ids=list(range(N_CORES)), trace=trace
    )
    out = np.concatenate([res.results[c]["out"] for c in range(N_CORES)], axis=0)
    out = out.astype(np.float32)  # widen fp16 device output (lossless)
    return out, res


def kernel(**inputs) -> np.ndarray:
    out, _ = _run(inputs, trace=False)
    return out
